# revision 13
# baseline (speedup 1.0000x reference)
"""GQA attention block (B=2,S=2048,D=4096,H=32,KV=8,HD=128) on 8 TRN2 NeuronCores.

Sharding: 8-way tensor parallel over heads. Core c owns kv-head c and q-heads
4c..4c+3 (wq/wk/wv column-sharded, wo row-sharded). The full-width Q/K
layernorms need cross-core mean/var, done with one tiny (64KB) on-device
AllReduce of per-token partial sums. Each core emits a partial [T,D] output
(its wo row-slice contribution); the host sums the 8 partials.

Device pipeline per core (all matmuls bf16, f32 accumulation):
  1. QKV projection (x^T chunks stationary, weight slices moving) + LN stats
  2. AllReduce stats -> mu/rstd; apply LN + RoPE; PE-transpose q,k to [hd,t]
  3. Attention per (b,h): scores^T = k_tile^T q (one orientation only),
     exp on ACT, attn@V with v stationary, softmax sums via ones-matmul,
     normalization folded into the psum->sbuf copyback
  4. Output projection into partial [T,D]
"""

import numpy as np
import ml_dtypes

import concourse.bass as bass
import concourse.mybir as mybir
import concourse.tile as tile
from concourse import bacc
from concourse import bass_utils
from concourse.bass import ts, ds
from concourse.masks import make_identity

BF16 = mybir.dt.bfloat16
F32 = mybir.dt.float32
AF = mybir.ActivationFunctionType
ALU = mybir.AluOpType
AX = mybir.AxisListType

B, S, D = 2, 2048, 4096
T = B * S                 # 4096 tokens
H, KV, HD = 32, 8, 128
NCORES = 8
HQ = H // NCORES          # 4 q heads per core
EQ = HQ * HD              # 512
NT = T // 128             # 32 token tiles
ND = D // 128             # 32 contraction chunks
ST = S // 128             # 16 seq tiles per batch
EPS = 1e-5
SHIFT = 12.0              # constant softmax shift (scores verified < ~8)

PROFILE = False
LAST_EXEC_NS = None
LAST_TRACE_DIR = None
_CACHE = {}


def _build():
    if "nc" in _CACHE:
        return _CACHE["nc"]
    nc = bacc.Bacc("TRN2", target_bir_lowering=False, debug=False,
                   num_devices=NCORES)

    xT_d = nc.dram_tensor("xT", [128, ND, T], BF16, kind="ExternalInput")
    wqT_d = nc.dram_tensor("wqT", [128, ND, EQ], BF16, kind="ExternalInput")
    wkT_d = nc.dram_tensor("wkT", [128, ND, HD], BF16, kind="ExternalInput")
    wvT_d = nc.dram_tensor("wvT", [128, ND, HD], BF16, kind="ExternalInput")
    woT_d = nc.dram_tensor("woT", [128, HQ, D], BF16, kind="ExternalInput")
    cosq_d = nc.dram_tensor("cosq", [T, HQ, 64, 2], BF16, kind="ExternalInput")
    sinq_d = nc.dram_tensor("sinq", [T, HQ, 64, 2], BF16, kind="ExternalInput")
    cosk_d = nc.dram_tensor("cosk", [T, 64, 2], BF16, kind="ExternalInput")
    sink_d = nc.dram_tensor("sink", [T, 64, 2], BF16, kind="ExternalInput")
    qw_d = nc.dram_tensor("qw", [1, EQ], F32, kind="ExternalInput")
    qb_d = nc.dram_tensor("qb", [1, EQ], F32, kind="ExternalInput")
    kw_d = nc.dram_tensor("kw", [1, HD], F32, kind="ExternalInput")
    kb_d = nc.dram_tensor("kb", [1, HD], F32, kind="ExternalInput")
    out_d = nc.dram_tensor("out", [T, D], F32, kind="ExternalOutput")

    with tile.TileContext(nc) as tc:
        _emit(nc, tc, xT_d, wqT_d, wkT_d, wvT_d, woT_d,
              cosq_d, sinq_d, cosk_d, sink_d, qw_d, qb_d, kw_d, kb_d, out_d)
    nc.compile()
    _CACHE["nc"] = nc
    return nc


def _emit(nc, tc, xT_d, wqT_d, wkT_d, wvT_d, woT_d,
          cosq_d, sinq_d, cosk_d, sink_d, qw_d, qb_d, kw_d, kb_d, out_d):
    from contextlib import ExitStack

    ctx = ExitStack()
    with ctx:
        cpool = ctx.enter_context(tc.tile_pool(name="cpool", bufs=1))
        persist = ctx.enter_context(tc.tile_pool(name="persist", bufs=1))

        # ---- constants ----
        ident = cpool.tile([128, 128], BF16, name="ident")
        make_identity(nc, ident[:])
        ones_r = cpool.tile([1, 128], F32, name="ones_r")     # K=1 bcast lhsT
        nc.vector.memset(ones_r[:], 1.0)
        ones_c = cpool.tile([128, 1], BF16, name="ones_c")    # partition-sum lhsT
        nc.vector.memset(ones_c[:], 1.0)
        eps_c = cpool.tile([128, 1], F32, name="eps_c")
        nc.vector.memset(eps_c[:], EPS)
        shift_c = cpool.tile([128, 1], F32, name="shift_c")
        nc.vector.memset(shift_c[:], -SHIFT)

        qw_sb = cpool.tile([1, EQ], F32, name="qw_sb")
        qb_sb = cpool.tile([1, EQ], F32, name="qb_sb")
        kw_sb = cpool.tile([1, HD], F32, name="kw_sb")
        kb_sb = cpool.tile([1, HD], F32, name="kb_sb")
        nc.sync.dma_start(qw_sb[:], qw_d.ap())
        nc.sync.dma_start(qb_sb[:], qb_d.ap())
        nc.sync.dma_start(kw_sb[:], kw_d.ap())
        nc.sync.dma_start(kb_sb[:], kb_d.ap())

        # broadcast LN vectors across partitions via K=1 matmul
        qwB = cpool.tile([128, HQ, 64, 2], F32, name="qwB")
        qbB = cpool.tile([128, HQ, 64, 2], F32, name="qbB")
        kwB = cpool.tile([128, 64, 2], F32, name="kwB")
        kbB = cpool.tile([128, 64, 2], F32, name="kbB")

        # persistent activations
        xq_raw = persist.tile([128, NT, HQ, 64, 2], BF16, name="xq_raw")
        xk_raw = persist.tile([128, NT, 64, 2], BF16, name="xk_raw")
        v_s = persist.tile([128, NT, HD], BF16, name="v_s")
        stats_s = persist.tile([128, NT, 4], F32, name="stats_s")
        stats_g = persist.tile([128, NT, 4], F32, name="stats_g")
        qT_s = persist.tile([128, HQ, T], BF16, name="qT_s")
        kT_s = persist.tile([128, T], BF16, name="kT_s")

        mu_q = cpool.tile([128, NT], F32, name="mu_q")
        rstd_q = cpool.tile([128, NT], F32, name="rstd_q")
        mu_k = cpool.tile([128, NT], F32, name="mu_k")
        rstd_k = cpool.tile([128, NT], F32, name="rstd_k")

        def flat2(ap):  # flatten all free dims -> [P, prod(free)]
            n = len(ap.shape)
            if n == 2:
                return ap
            names = " ".join(f"d{i}" for i in range(n - 1))
            return ap.rearrange(f"p {names} -> p ({names})")

        # ================= phase 1: QKV projection + stats =================
        with tc.tile_pool(name="p1w", bufs=1) as p1w, \
             tc.tile_pool(name="p1x", bufs=3) as p1x, \
             tc.tile_pool(name="p1s", bufs=2) as p1s, \
             tc.tile_pool(name="ps1", bufs=1, space="PSUM") as ps1:

            # LN vector broadcasts (one-time, share psum tag with psq)
            for bcsrc, bcdst, wid in ((qw_sb, qwB, EQ), (qb_sb, qbB, EQ),
                                      (kw_sb, kwB, HD), (kb_sb, kbB, HD)):
                ps_bc = ps1.tile([128, wid], F32, tag="psq", bufs=2)
                nc.tensor.matmul(ps_bc[:], lhsT=ones_r[:], rhs=bcsrc[:],
                                 start=True, stop=True)
                nc.scalar.copy(flat2(bcdst[:]), ps_bc[:])

            wq_s = p1w.tile([128, ND, EQ], BF16, name="wq_s")
            wk_s = p1w.tile([128, ND, HD], BF16, name="wk_s")
            wv_s = p1w.tile([128, ND, HD], BF16, name="wv_s")
            nc.sync.dma_start(wq_s[:], wqT_d.ap())
            nc.sync.dma_start(wk_s[:], wkT_d.ap())
            nc.sync.dma_start(wv_s[:], wvT_d.ap())

            for ti in range(NT):
                x_t = p1x.tile([128, ND, 128], BF16, tag="x_t", bufs=3)
                nc.sync.dma_start(x_t[:], xT_d.ap()[:, :, ts(ti, 128)])

                psq = ps1.tile([128, EQ], F32, tag="psq", bufs=2)
                psk = ps1.tile([128, HD], F32, tag="psk", bufs=2)
                psv = ps1.tile([128, HD], F32, tag="psv", bufs=2)
                for j in range(ND):
                    lx = x_t[:, j, :]
                    nc.tensor.matmul(psq[:], lhsT=lx, rhs=wq_s[:, j, :],
                                     start=(j == 0), stop=(j == ND - 1))
                for j in range(ND):
                    lx = x_t[:, j, :]
                    nc.tensor.matmul(psk[:], lhsT=lx, rhs=wk_s[:, j, :],
                                     start=(j == 0), stop=(j == ND - 1))
                for j in range(ND):
                    lx = x_t[:, j, :]
                    nc.tensor.matmul(psv[:], lhsT=lx, rhs=wv_s[:, j, :],
                                     start=(j == 0), stop=(j == ND - 1))

                # copybacks (raw, pre-LN) to bf16
                nc.scalar.copy(flat2(xq_raw[:, ti]), psq[:])
                nc.scalar.copy(flat2(xk_raw[:, ti]), psk[:])
                nc.scalar.copy(v_s[:, ti, :], psv[:])

                # per-token partial sums (f32, straight from psum)
                scrap = p1s.tile([128, EQ], BF16, tag="scrap", bufs=2)
                nc.vector.tensor_reduce(
                    out=stats_s[:, ti, 0:1], in_=psq[:], axis=AX.X,
                    op=ALU.add)
                nc.scalar.activation(scrap[:], psq[:], AF.Square,
                                     accum_out=stats_s[:, ti, 1:2])
                scrapk = p1s.tile([128, HD], BF16, tag="scrapk", bufs=2)
                nc.vector.tensor_reduce(
                    out=stats_s[:, ti, 2:3], in_=psk[:], axis=AX.X,
                    op=ALU.add)
                nc.scalar.activation(scrapk[:], psk[:], AF.Square,
                                     accum_out=stats_s[:, ti, 3:4])

        # ================= AllReduce of LN stats =================
        with tc.tile_pool(name="ardram", bufs=1, space="DRAM") as ardram:
            ar_in = ardram.tile([128, NT, 4], F32, name="ar_in")
            ar_out = ardram.tile([128, NT, 4], F32, name="ar_out",
                                 addr_space="Shared")
            nc.gpsimd.dma_start(ar_in[:], stats_s[:])
            nc.gpsimd.collective_compute(
                "AllReduce", ALU.add,
                replica_groups=[list(range(NCORES))],
                ins=[ar_in.opt()], outs=[ar_out.opt()])
            nc.gpsimd.dma_start(stats_g[:], ar_out[:])

        # mu/rstd for q (E=4096) and k (E=1024)
        def stat(k):
            return stats_g[:, :, k:k + 1].rearrange("p t s -> p (t s)")

        tmp_a = cpool.tile([128, NT], F32, name="tmp_a")
        tmp_b = cpool.tile([128, NT], F32, name="tmp_b")
        for (mu_t, rstd_t, s0, s1, e_full) in (
                (mu_q, rstd_q, 0, 1, D), (mu_k, rstd_k, 2, 3, KV * HD)):
            nc.vector.tensor_scalar_mul(mu_t[:], stat(s0), 1.0 / e_full)
            nc.vector.tensor_scalar_mul(tmp_a[:], stat(s1), 1.0 / e_full)
            nc.vector.tensor_mul(tmp_b[:], mu_t[:], mu_t[:])
            nc.vector.tensor_sub(tmp_a[:], tmp_a[:], tmp_b[:])
            nc.scalar.activation(tmp_b[:], tmp_a[:], AF.Sqrt, bias=eps_c[:])
            nc.vector.reciprocal(rstd_t[:], tmp_b[:])

        # ================= phase 2: LN + RoPE + transpose =================
        with tc.tile_pool(name="p2", bufs=2) as p2, \
             tc.tile_pool(name="ps2", bufs=2, space="PSUM") as ps2:
            for ti in range(NT):
                cosq_t = p2.tile([128, HQ, 64, 2], BF16, tag="cosq", bufs=2)
                sinq_t = p2.tile([128, HQ, 64, 2], BF16, tag="sinq", bufs=2)
                cosk_t = p2.tile([128, 64, 2], BF16, tag="cosk", bufs=2)
                sink_t = p2.tile([128, 64, 2], BF16, tag="sink", bufs=2)
                nc.sync.dma_start(cosq_t[:], cosq_d.ap()[ts(ti, 128)])
                nc.sync.dma_start(sinq_t[:], sinq_d.ap()[ts(ti, 128)])
                nc.sync.dma_start(cosk_t[:], cosk_d.ap()[ts(ti, 128)])
                nc.sync.dma_start(sink_t[:], sink_d.ap()[ts(ti, 128)])

                for (nh, raw, wB, bB, cos_t, sin_t, mu_t, rstd_t, tT) in (
                        (HQ, xq_raw[:, ti], qwB, qbB, cosq_t, sinq_t,
                         mu_q, rstd_q, qT_s),
                        (1, xk_raw[:, ti], kwB, kbB, cosk_t, sink_t,
                         mu_k, rstd_k, kT_s)):
                    sfx = "q" if nh == HQ else "k"
                    shp = [128, nh, 64, 2] if nh > 1 else [128, 64, 2]
                    A_t = p2.tile(shp, F32, tag=f"A{sfx}", bufs=2)
                    B_t = p2.tile(shp, F32, tag=f"B{sfx}", bufs=2)
                    xn_t = p2.tile(shp, BF16, tag=f"xn{sfx}", bufs=2)
                    x2_t = p2.tile(shp, BF16, tag=f"x2{sfx}", bufs=2)
                    rot_t = p2.tile(shp, BF16, tag=f"rot{sfx}", bufs=2)
                    rp_t = p2.tile(shp, BF16, tag=f"rp{sfx}", bufs=2)

                    mu_ap = mu_t[:, ti:ti + 1]
                    rstd_ap = rstd_t[:, ti:ti + 1]
                    # A = w * rstd ; B = b - mu*A ; xn = raw*A + B
                    nc.vector.tensor_scalar_mul(A_t[:], wB[:], rstd_ap)
                    nc.vector.tensor_scalar_mul(B_t[:], A_t[:], mu_ap)
                    nc.vector.tensor_sub(B_t[:], bB[:], B_t[:])
                    nc.vector.tensor_mul(xn_t[:], raw, A_t[:])
                    nc.vector.tensor_add(x2_t[:], xn_t[:], B_t[:])
                    # rope: out = xn*cos + rot(xn)*sin_signed
                    if nh > 1:
                        nc.vector.tensor_copy(rot_t[:, :, :, 0:1],
                                              x2_t[:, :, :, 1:2])
                        nc.vector.tensor_copy(rot_t[:, :, :, 1:2],
                                              x2_t[:, :, :, 0:1])
                    else:
                        nc.vector.tensor_copy(rot_t[:, :, 0:1],
                                              x2_t[:, :, 1:2])
                        nc.vector.tensor_copy(rot_t[:, :, 1:2],
                                              x2_t[:, :, 0:1])
                    nc.vector.tensor_mul(xn_t[:], x2_t[:], cos_t[:])
                    nc.vector.tensor_mul(rot_t[:], rot_t[:], sin_t[:])
                    nc.vector.tensor_add(rp_t[:], xn_t[:], rot_t[:])

                    for h in range(nh):
                        tp_ps = ps2.tile([128, 128], BF16, tag="tp", bufs=2)
                        src = rp_t[:, h] if nh > 1 else rp_t[:]
                        nc.tensor.transpose(tp_ps[:], src, ident[:])
                        if nh > 1:
                            nc.scalar.copy(tT[:, h, ts(ti, 128)], tp_ps[:])
                        else:
                            nc.scalar.copy(tT[:, ts(ti, 128)], tp_ps[:])

        # ================= phase 3: attention =================
        with tc.tile_pool(name="p34", bufs=1) as p34:
            oT_s = p34.tile([128, HQ, T], BF16, name="oT_s")
            woT_s = p34.tile([128, HQ, D], BF16, name="woT_s")
            nc.sync.dma_start(woT_s[:], woT_d.ap())
            _attn(nc, tc, qT_s, kT_s, v_s, oT_s, ones_r, ones_c, shift_c)
            _wo_proj(nc, tc, oT_s, woT_s, out_d)


def _attn(nc, tc, qT_s, kT_s, v_s, oT_s, ones_r, ones_c, shift_c):
    with tc.tile_pool(name="p3", bufs=1) as p3, \
         tc.tile_pool(name="ps3", bufs=1, space="PSUM") as ps3:
            for b in range(B):
                for h in range(HQ):
                    for qb in range(S // 512):
                        q_ap = qT_s[:, h, ds(b * S + qb * 512, 512)]
                        psV = ps3.tile([128, 512], F32, tag="psV", bufs=2)
                        psSum = ps3.tile([1, 512], F32, tag="psSum", bufs=2)
                        for kt in range(ST):
                            psB = ps3.tile([128, 512], F32, tag="psB", bufs=2)
                            nc.tensor.matmul(
                                psB[:], lhsT=kT_s[:, ds(b * S + kt * 128, 128)],
                                rhs=q_ap, start=True, stop=True)
                            attnT = p3.tile([128, 512], BF16, tag="attnT",
                                            bufs=3)
                            nc.scalar.activation(attnT[:], psB[:], AF.Exp,
                                                 bias=shift_c[:])
                            nc.tensor.matmul(
                                psV[:], lhsT=v_s[:, b * ST + kt, :],
                                rhs=attnT[:], start=(kt == 0),
                                stop=(kt == ST - 1))
                            nc.tensor.matmul(
                                psSum[:], lhsT=ones_c[:], rhs=attnT[:],
                                start=(kt == 0), stop=(kt == ST - 1))
                        recipR = p3.tile([1, 512], F32, tag="recipR", bufs=2)
                        nc.vector.reciprocal(recipR[:], psSum[:])
                        bc_ps = ps3.tile([128, 512], F32, tag="psB", bufs=2)
                        nc.tensor.matmul(bc_ps[:], lhsT=ones_r[:],
                                         rhs=recipR[:], start=True, stop=True)
                        bc_sb = p3.tile([128, 512], F32, tag="bc_sb", bufs=2)
                        nc.scalar.copy(bc_sb[:], bc_ps[:])
                        nc.vector.tensor_mul(
                            oT_s[:, h, ds(b * S + qb * 512, 512)],
                            psV[:], bc_sb[:])


def _wo_proj(nc, tc, oT_s, woT_s, out_d):
    # ================= phase 4: output projection =================
    with tc.tile_pool(name="p4", bufs=1) as p4, \
         tc.tile_pool(name="ps4", bufs=1, space="PSUM") as ps4:
        for ti in range(NT):
            for half in range(2):
                psO = ps4.tile([128, 4, 512], F32, tag="psO", bufs=2)
                for nb in range(4):
                    for h in range(HQ):
                        nc.tensor.matmul(
                            psO[:, nb],
                            lhsT=oT_s[:, h, ts(ti, 128)],
                            rhs=woT_s[:, h,
                                      ds(half * 2048 + nb * 512, 512)],
                            start=(h == 0), stop=(h == HQ - 1))
                outst = p4.tile([128, 4, 512], F32, tag="outst", bufs=3)
                nc.vector.tensor_copy(outst[:], psO[:])
                nc.sync.dma_start(
                    out_d.ap()[ts(ti, 128), ds(half * 2048, 2048)],
                    outst[:].rearrange("p a b -> p (a b)"))


def _host_inputs(x, freqs_cis, wq, wk, wv, wo, q_norm_w, q_norm_b,
                 k_norm_w, k_norm_b):
    bf = ml_dtypes.bfloat16
    f32 = np.float32
    x = np.asarray(x, f32)
    freqs_cis = np.asarray(freqs_cis, f32)
    wq = np.asarray(wq, f32)
    wk = np.asarray(wk, f32)
    wv = np.asarray(wv, f32)
    wo = np.asarray(wo, f32)
    q_norm_w = np.asarray(q_norm_w, f32)
    q_norm_b = np.asarray(q_norm_b, f32)
    k_norm_w = np.asarray(k_norm_w, f32)
    k_norm_b = np.asarray(k_norm_b, f32)

    xf = np.ascontiguousarray(x.reshape(T, D))
    xT_r = np.ascontiguousarray(
        xf.T.reshape(ND, 128, T).transpose(1, 0, 2)).astype(bf)

    cos = freqs_cis[:, :, 0]          # [S, 64]
    sin = freqs_cis[:, :, 1]
    cos2 = np.concatenate([cos] * B, 0)   # [T, 64]
    sin2 = np.concatenate([sin] * B, 0)
    cosP = np.stack([cos2, cos2], -1)     # [T, 64, 2]
    sinP = np.stack([-sin2, sin2], -1)    # [T, 64, 2]
    cosq = np.ascontiguousarray(
        np.broadcast_to(cosP[:, None], (T, HQ, 64, 2))).astype(bf)
    sinq = np.ascontiguousarray(
        np.broadcast_to(sinP[:, None], (T, HQ, 64, 2))).astype(bf)
    cosk = np.ascontiguousarray(cosP).astype(bf)
    sink = np.ascontiguousarray(sinP).astype(bf)

    scale = 1.0 / np.sqrt(np.float32(HD))
    in_maps = []
    for c in range(NCORES):
        wq_c = wq[c * EQ:(c + 1) * EQ]           # [512, D]
        wk_c = wk[c * HD:(c + 1) * HD]           # [128, D]
        wv_c = wv[c * HD:(c + 1) * HD]
        wo_c = wo[:, c * EQ:(c + 1) * EQ]        # [D, 512]
        wqT_r = np.ascontiguousarray(
            wq_c.T.reshape(ND, 128, EQ).transpose(1, 0, 2)).astype(bf)
        wkT_r = np.ascontiguousarray(
            wk_c.T.reshape(ND, 128, HD).transpose(1, 0, 2)).astype(bf)
        wvT_r = np.ascontiguousarray(
            wv_c.T.reshape(ND, 128, HD).transpose(1, 0, 2)).astype(bf)
        woT_r = np.ascontiguousarray(
            wo_c.T.reshape(HQ, 128, D).transpose(1, 0, 2)).astype(bf)
        qw_c = (q_norm_w[c * EQ:(c + 1) * EQ] * scale).astype(f32).reshape(1, EQ)
        qb_c = (q_norm_b[c * EQ:(c + 1) * EQ] * scale).astype(f32).reshape(1, EQ)
        kw_c = k_norm_w[c * HD:(c + 1) * HD].astype(f32).reshape(1, HD)
        kb_c = k_norm_b[c * HD:(c + 1) * HD].astype(f32).reshape(1, HD)
        in_maps.append({
            "xT": xT_r, "wqT": wqT_r, "wkT": wkT_r, "wvT": wvT_r,
            "woT": woT_r, "cosq": cosq, "sinq": sinq, "cosk": cosk,
            "sink": sink, "qw": qw_c, "qb": qb_c, "kw": kw_c, "kb": kb_c,
        })
    return in_maps


def _run_profiled(nc, in_maps):
    """bass2jax execute wrapped in an NRT profile capture; returns
    (results, max exec_time_ns across cores, trace_dir)."""
    import ctypes
    import glob
    import tempfile

    import jax
    from concourse import bass2jax
    import gauge.profiler
    from concourse.bass_utils import FishPath

    lib = ctypes.CDLL("/opt/axon/libaxon_pjrt.so")
    if not hasattr(lib, "axon_start_nrt_profile"):
        results = bass2jax.run_bass_via_pjrt(nc, in_maps, n_cores=NCORES)
        return results, None, None
    lib.axon_start_nrt_profile.argtypes = [ctypes.POINTER(ctypes.c_int64),
                                           ctypes.c_size_t]
    lib.axon_start_nrt_profile.restype = ctypes.c_int64
    lib.axon_stop_nrt_profile.argtypes = [ctypes.c_char_p]
    lib.axon_stop_nrt_profile.restype = ctypes.c_int64

    jax.devices()
    neff_dir = tempfile.mkdtemp(prefix="bassprof_")
    rc = lib.axon_start_nrt_profile(None, 0)
    if rc != 0:
        raise RuntimeError(f"axon_start_nrt_profile rc={rc}")
    try:
        results = bass2jax.run_bass_via_pjrt(nc, in_maps, n_cores=NCORES)
    finally:
        n = lib.axon_stop_nrt_profile(neff_dir.encode())
        print(f"profile: {n} ntff file(s) in {neff_dir}")
    ntffs = glob.glob(neff_dir + "/*_body*.ntff")
    if not ntffs:
        return results, None, None
    profile = gauge.profiler.Profile(
        profile_path=FishPath(neff_dir), kernel_dev_mode=True,
        profile_on_exit=False, bass_kernel=nc.m,
        offline_processing=True, fname="*_body*")
    exec_ns = None
    try:
        prs = profile.to_perfetto(model_index=list(range(NCORES)))
        times = [pr.exec_time_ns for pr in prs if pr.exec_time_ns]
        exec_ns = max(times) if times else None
    except Exception as e:  # profile parse best-effort
        print("profile parse failed:", e)
    return results, exec_ns, neff_dir


def kernel(x, freqs_cis, wq, wk, wv, wo, q_norm_w, q_norm_b,
           k_norm_w, k_norm_b):
    global LAST_EXEC_NS, LAST_TRACE_DIR
    nc = _build()
    in_maps = _host_inputs(x, freqs_cis, wq, wk, wv, wo,
                           q_norm_w, q_norm_b, k_norm_w, k_norm_b)
    if PROFILE:
        results, LAST_EXEC_NS, LAST_TRACE_DIR = _run_profiled(nc, in_maps)
    else:
        res = bass_utils.run_bass_kernel_spmd(
            nc, in_maps, core_ids=list(range(NCORES)))
        results = res.results
        LAST_EXEC_NS = res.exec_time_ns
    acc = np.zeros((T, D), np.float32)
    for r in results:
        acc += np.asarray(r["out"], np.float32)
    return acc.reshape(B, S, D)
